# revision 3
# baseline (speedup 1.0000x reference)
"""Distributed Trainium2 kernel for AdvancedMultiHeadAttention (fp8 DoubleRow).

B=2, T=2048, C=1024, H=16 heads, D=64. Causal SDPA with RoPE.
Sharding: data-parallel over batch (cores 0-3 = batch 0, 4-7 = batch 1),
tensor-parallel over heads within each group (4 heads/core). Wo is
row-sharded; partial sums are reduced with four bf16 ReduceScatters per
4-core replica group (one per 512-query span, overlapped), host concat.

Numerics: big matmuls run as fp8e4m3 DoubleRow (2x128 contraction/pass,
0.5 cyc/col). Projections and S=QK^T use hi+lo residual splitting (3
terms, error ~fp8^2); AV uses single-fp8 attention weights with hi+lo V
(2 terms). Wo stays bf16. End-to-end rel err ~1.3e-2 (tol 2e-2).

Scales: Wq,Wk x256 on host, cos/sin tables /16 -> Q,K stored as 16*q_rot.
exp(scale=1/2048, bias=-3ln2) -> at = e^u/8 in [0,30] (fp8e4 max 240).
Wv x16 -> V stored 16*v; OT = 16*attn_out; Wo/16 on host -> psY = y.

Scheduling: projection / V-projection / Wo matmul groups are interleaved
into the attention kt loop as fillers so the PE never sits behind the
Act-engine softmax. PSUM: tag "eo" (2 banks, proj+V), tag "s" (4 banks,
S-pairs + Wo), o0/o1 (2 banks, AV accumulators for one head-pair).

Dual-fp8 LdWeights ISA restrictions handled: no semaphore waits on
ldweights (wait-move pass disabled, PE fuse_nops disabled, waits hoisted
to EventSemaphore), and pair-dim stride of every DR stationary AP is
16-byte aligned (V tile padded to 68 cols/head).
"""

import sys

sys.path.insert(0, "/opt/trn_rl_repo")

import math

import ml_dtypes
import numpy as np

B, T, C = 2, 2048, 1024
H, D = 16, 64
NCORES = 8
HLOC = 4            # heads per core
CLOC = HLOC * D     # 256 attention dims per core
NSPAN = T // 512    # 4 query spans
VP = 68             # padded V columns per head (pair stride 4*68 % 16 == 0)
RG = [[0, 1, 2, 3], [4, 5, 6, 7]]

_CACHE = {}


def _build_nc(rs=True, reps=1):
    import concourse.bacc as bacc
    import concourse.mybir as mybir
    import concourse.tile as tile

    f32 = mybir.dt.float32
    bf16 = mybir.dt.bfloat16
    fp8 = mybir.dt.float8e4
    AF = mybir.ActivationFunctionType
    DR = mybir.MatmulPerfMode.DoubleRow
    ALU = mybir.AluOpType
    ESC = 0.125 / 256.0
    EBIAS = -3.0 * math.log(2.0)

    nc = bacc.Bacc("TRN2", target_bir_lowering=False, debug=False,
                   num_devices=NCORES)
    # Dual-fp8 (DoubleRow) InstLdweights cannot carry semaphore waits
    # (walrus s3_lw_dual_fp8_restrictions). Keep waits off ldweights.
    nc.move_matmul_waits_to_ldweights = lambda: None
    _orig_fuse_nops = type(nc).fuse_nops
    nc.fuse_nops = (lambda engine: None if engine == mybir.EngineType.PE
                    else _orig_fuse_nops(nc, engine))

    xT = nc.declare_dram_parameter("xT", [512, 8192], fp8, isOutput=False)
    wT = nc.declare_dram_parameter("wT", [512, 3072], fp8, isOutput=False)
    woT = nc.declare_dram_parameter("woT", [128, 2 * C], bf16, isOutput=False)
    csT = nc.declare_dram_parameter("csT", [128, 2 * T], bf16, isOutput=False)
    bandT = nc.declare_dram_parameter("band", [128, 128], fp8, isOutput=False)
    outx = nc.declare_dram_parameter("out", [512, C], bf16, isOutput=True)

    with tile.TileContext(nc) as tc:
        with (
            tc.tile_pool(name="cst", bufs=1) as cst,
            tc.tile_pool(name="work", bufs=1) as work,
            tc.tile_pool(name="ps", bufs=1, space="PSUM") as ps,
            tc.tile_pool(name="dram", bufs=1, space="DRAM") as dram,
        ):
            # ---- constant loads: merged hi/lo tiles, one DMA each ----
            w2 = [cst.tile([128, 2 * 1536], fp8, tag=f"w2{p}", name=f"w2{p}")
                  for p in range(4)]
            x2 = [[cst.tile([128, 2 * 2048], fp8, tag=f"x2{p}_{s}",
                            name=f"x2{p}_{s}") for s in range(2)]
                  for p in range(4)]
            cs_sb = cst.tile([128, 2 * T], bf16, tag="cs", name="cs")
            for p in range(4):
                r = slice(p * 128, (p + 1) * 128)
                nc.sync.dma_start(w2[p][:], wT[r, :])
                nc.sync.dma_start(x2[p][0][:], xT[r, 0:4096])
            # cos/sin for span 0 right behind the first projection's data
            for spc in range(4):
                nc.sync.dma_start(cs_sb[:, spc * 512:(spc + 1) * 512],
                                  csT[:, spc * 512:(spc + 1) * 512])
                nc.sync.dma_start(cs_sb[:, T + spc * 512:T + (spc + 1) * 512],
                                  csT[:, T + spc * 512:T + (spc + 1) * 512])
            band_sb = cst.tile([128, 128], fp8, tag="band", name="band")
            nc.sync.dma_start(band_sb[:], bandT[:, :])
            for p in range(4):
                r = slice(p * 128, (p + 1) * 128)
                nc.sync.dma_start(x2[p][1][:], xT[r, 4096:8192])
            wo2_sb = cst.tile([128, 2 * C], bf16, tag="wo2", name="wo2")
            nc.sync.dma_start(wo2_sb[:], woT[:, :])
            wo_sb = [wo2_sb[:, i * C:(i + 1) * C] for i in range(2)]

            cos_sb = cs_sb[:, 0:T]
            sin_sb = cs_sb[:, T:2 * T]
            eb = cst.tile([128, 1], f32, tag="eb", name="eb")
            nc.vector.memset(eb[:], EBIAS)

            def w3(lo, p, c0, n):
                ofs = 768 if lo else 0
                return w2[p][:].rearrange(
                    "p (i c) -> p i c", i=2)[:, :, ofs + c0:ofs + c0 + n]

            def x3(lo, p, half, c0, n):
                ofs = 1024 if lo else 0
                return x2[p][half][:].rearrange(
                    "p (i c) -> p i c", i=2)[:, :, ofs + c0:ofs + c0 + n]

            for _rep in range(reps):
                QDh = work.tile([128, 2 * T], fp8, tag="qdh", name="qdh")
                QDl = work.tile([128, 2 * T], fp8, tag="qdl", name="qdl")
                KDh = work.tile([128, 2 * T], fp8, tag="kdh", name="kdh")
                KDl = work.tile([128, 2 * T], fp8, tag="kdl", name="kdl")
                OT2 = work.tile([128, 2 * T], bf16, tag="ot2", name="ot2")
                vah = [work.tile([128, 2 * HLOC * VP], fp8, tag=f"vah{p}",
                                 name=f"vah{p}") for p in range(8)]
                val = [work.tile([128, 2 * HLOC * VP], fp8, tag=f"val{p}",
                                 name=f"val{p}") for p in range(8)]

                def d3(tile_, h, c0, n):
                    return tile_[h * 32:(h + 1) * 32].rearrange(
                        "p (i t) -> p i t", i=2)[:, :, c0:c0 + n]

                def proj_half(sp, c0, pe, hx):
                    cofs = c0 + hx * 128
                    dst = pe[:, hx * 512:(hx + 1) * 512]
                    mm = 0
                    for p in range(4):
                        lw_h = w3(False, p, cofs, 128)
                        lw_l = w3(True, p, cofs, 128)
                        rx_h = x3(False, p, sp // 2, (sp % 2) * 512, 512)
                        rx_l = x3(True, p, sp // 2, (sp % 2) * 512, 512)
                        for lw, rx in ((lw_h, rx_h), (lw_h, rx_l),
                                       (lw_l, rx_h)):
                            nc.tensor.matmul(dst, lw, rx, start=(mm == 0),
                                             stop=(mm == 11), perf_mode=DR)
                            mm += 1

                def proj_mm(sp, c0, tag="eo", bufs=1):
                    """QKV projection matmuls for q (c0=0) or k (c0=256)."""
                    pe = ps.tile([128, 1024], f32, tag=tag, name="psEO",
                                 bufs=bufs)
                    proj_half(sp, c0, pe, 0)
                    proj_half(sp, c0, pe, 1)
                    return pe

                def rope_a(sp, pe):
                    qs = slice(sp * 512, (sp + 1) * 512)
                    psE = pe[:, 0:512]
                    t1 = work.tile([128, 512], bf16, tag="t1", name="t1", bufs=2)
                    t3 = work.tile([128, 512], bf16, tag="t3", name="t3", bufs=2)
                    with nc.allow_low_precision(reason="rope in bf16/fp8"):
                        nc.vector.tensor_mul(t1[:], psE, cos_sb[:, qs])
                        nc.vector.tensor_mul(t3[:], psE, sin_sb[:, qs])
                    return t1, t3

                def rope_b(sp, pe, DH, DL, t1, t3):
                    qs = slice(sp * 512, (sp + 1) * 512)
                    psO = pe[:, 512:1024]
                    t2 = work.tile([128, 512], bf16, tag="t2", name="t2", bufs=2)
                    t4 = work.tile([128, 512], bf16, tag="t4", name="t4", bufs=2)
                    TOP = work.tile([128, 512], bf16, tag="top", name="top",
                                    bufs=2)
                    BOT = work.tile([128, 512], bf16, tag="bot", name="bot",
                                    bufs=2)
                    with nc.allow_low_precision(reason="rope in bf16/fp8"):
                        nc.vector.tensor_mul(t2[:], psO, sin_sb[:, qs])
                        nc.vector.tensor_sub(TOP[:], t1[:], t2[:])
                        nc.vector.tensor_mul(t4[:], psO, cos_sb[:, qs])
                        nc.vector.tensor_add(BOT[:], t3[:], t4[:])
                        for blk, SRC in ((0, TOP), (1, BOT)):
                            csl = slice(blk * T + sp * 512,
                                        blk * T + (sp + 1) * 512)
                            nc.gpsimd.tensor_copy(DH[:, csl], SRC[:])
                            nc.vector.scalar_tensor_tensor(
                                DL[:, csl], DH[:, csl], -1.0, SRC[:],
                                ALU.mult, ALU.add)

                def rope_ev(sp, pe, DH, DL):
                    t1, t3 = rope_a(sp, pe)
                    rope_b(sp, pe, DH, DL, t1, t3)

                def v_quad(quad, tag="eo", bufs=1, evict_dve=False):
                    """V projection for tts 4q..4q+3 -> vaug pairs 2q, 2q+1."""
                    pv = ps.tile([128, 1024], f32, tag=tag, name="psV",
                                 bufs=bufs)
                    for pr in (quad * 2, quad * 2 + 1):
                        with nc.allow_low_precision(reason="ones col"):
                            o_h = vah[pr][:].rearrange("p (i d) -> p i d", i=8)
                            o_l = val[pr][:].rearrange("p (i d) -> p i d", i=8)
                            nc.gpsimd.memset(o_h[:, :, 64:65], 1.0)
                            nc.gpsimd.memset(o_l[:, :, 64:65], 0.0)
                    for ttl in range(4):
                        tt = quad * 4 + ttl
                        dst = pv[:, ttl * 256:(ttl + 1) * 256]
                        mm = 0
                        for p in range(4):
                            lx_h = x3(False, p, tt // 8, (tt % 8) * 128, 128)
                            lx_l = x3(True, p, tt // 8, (tt % 8) * 128, 128)
                            rw_h = w3(False, p, 512, 256)
                            rw_l = w3(True, p, 512, 256)
                            for lx, rw in ((lx_h, rw_h), (lx_l, rw_h),
                                           (lx_h, rw_l)):
                                nc.tensor.matmul(dst, lx, rw, start=(mm == 0),
                                                 stop=(mm == 11), perf_mode=DR)
                                mm += 1
                        pr = quad * 2 + ttl // 2
                        i = tt % 2
                        src = dst.rearrange("p (h d) -> p h d", h=HLOC)
                        vh_dst = vah[pr][:].rearrange(
                            "p (i h d) -> p i h d", i=2, h=HLOC)[:, i, :, 0:64]
                        vl_dst = val[pr][:].rearrange(
                            "p (i h d) -> p i h d", i=2, h=HLOC)[:, i, :, 0:64]
                        with nc.allow_low_precision(reason="v fp8 hi/lo"):
                            nc.scalar.copy(vh_dst, src)
                            nc.vector.scalar_tensor_tensor(
                                vl_dst, vh_dst, -1.0, src, ALU.mult, ALU.add)

                def wo_tt(c, ttl):
                    yb = _ybs[c]
                    if True:
                        tt = c * 4 + ttl
                        psY = ps.tile([128, 1024], f32,
                                      tag=("eo" if c < 3 else "s"), name="psY",
                                      bufs=(1 if c < 3 else 2))
                        for cs in range(2):
                            for s2 in range(2):
                                nc.tensor.matmul(
                                    psY[:, cs * 512:(cs + 1) * 512],
                                    OT2[:, s2 * T + tt * 128:
                                        s2 * T + (tt + 1) * 128],
                                    wo_sb[s2][:, cs * 512:(cs + 1) * 512],
                                    start=(s2 == 0), stop=(s2 == 1))
                        ysb = work.tile([128, C], bf16, tag="ysb", name="ysb",
                                        bufs=2)
                        nc.scalar.copy(ysb[:], psY[:])
                        nc.sync.dma_start(yb[ttl * 128:(ttl + 1) * 128, :],
                                          ysb[:])

                def rs_chunk(c):
                    import concourse.mybir as mybir_
                    yb = _ybs[c]
                    if rs:
                        rst = dram.tile([128, C], bf16, tag=f"rs{c}_{_rep}",
                                        name=f"rs{c}_{_rep}")
                        nc.gpsimd.collective_compute(
                            "ReduceScatter", mybir_.AluOpType.add,
                            replica_groups=RG,
                            ins=[yb[:].opt()], outs=[rst[:].opt()])
                        nc.sync.dma_start(outx[c * 128:(c + 1) * 128, :],
                                          rst[:])
                    else:
                        core_r = 0
                        nc.sync.dma_start(outx[c * 128:(c + 1) * 128, :],
                                          yb[core_r * 128:(core_r + 1) * 128, :])

                _ybs = [dram.tile([512, C], bf16, tag=f"yb{c}_{_rep}",
                                  name=f"yb{c}_{_rep}") for c in range(4)]

                def attn_span(sp, fillers):
                    qs = slice(sp * 512, (sp + 1) * 512)
                    npair = 2 * (sp + 1)
                    fidx = [0]

                    def fill():
                        if fidx[0] < len(fillers):
                            f = fillers[fidx[0]]
                            fidx[0] += 1
                            if f is not None:
                                f()

                    for hp in (0, 1):
                        psO = [ps.tile([65, 512], f32, tag=f"o{i}",
                                       name=f"psAcc{i}") for i in (0, 1)]
                        prev = None

                        def flush(prev):
                            ats, pktp = prev
                            for i in (0, 1):
                                h = 2 * hp + i
                                rhs = ats[i][:].rearrange("p (i t) -> p i t",
                                                          i=2)
                                lw_h = vah[pktp][:].rearrange(
                                    "p (i h d) -> p i h d", i=2,
                                    h=HLOC)[:, :, h, 0:65]
                                lw_l = val[pktp][:].rearrange(
                                    "p (i h d) -> p i h d", i=2,
                                    h=HLOC)[:, :, h, 0:65]
                                nc.tensor.matmul(psO[i][:], lw_h, rhs,
                                                 start=(pktp == 0), stop=False,
                                                 perf_mode=DR)
                                nc.tensor.matmul(psO[i][:], lw_l, rhs,
                                                 start=False,
                                                 stop=(pktp == npair - 1),
                                                 perf_mode=DR)

                        for ktp in range(npair):
                            ats = []
                            for i in (0, 1):
                                h = 2 * hp + i
                                psS = ps.tile([128, 1024], f32, tag="s",
                                              name=f"psS{i}", bufs=2)
                                for j, kt in enumerate((2 * ktp, 2 * ktp + 1)):
                                    dst = psS[:, j * 512:(j + 1) * 512]
                                    kh = d3(KDh, h, kt * 128, 128)
                                    kl = d3(KDl, h, kt * 128, 128)
                                    qh = d3(QDh, h, sp * 512, 512)
                                    ql = d3(QDl, h, sp * 512, 512)
                                    for mm, (lk, rq) in enumerate(
                                            ((kh, qh), (kh, ql), (kl, qh))):
                                        nc.tensor.matmul(
                                            dst, lk, rq, start=(mm == 0),
                                            stop=(mm == 2), perf_mode=DR,
                                            tile_position=(h * 32, 0))
                                at = work.tile([128, 1024], fp8, tag=f"at{i}",
                                               name=f"at{i}", bufs=8)
                                if ktp < 2 * sp:
                                    nc.scalar.activation(at[:], psS[:], AF.Exp,
                                                         scale=ESC, bias=eb[:])
                                else:
                                    for j in (0, 1):
                                        d = (2 * ktp + j) * 128 - sp * 512
                                        c0 = j * 512
                                        if d > 0:
                                            nc.gpsimd.memset(
                                                at[:, c0:c0 + d], 0.0)
                                        nc.scalar.activation(
                                            at[:, c0 + d:c0 + 512],
                                            psS[:, c0 + d:c0 + 512],
                                            AF.Exp, scale=ESC, bias=eb[:])
                                        with nc.allow_low_precision(
                                                reason="mask"):
                                            nc.gpsimd.tensor_mul(
                                                at[:, c0 + d:c0 + d + 128],
                                                at[:, c0 + d:c0 + d + 128],
                                                band_sb[:])
                                ats.append(at)
                            fill()
                            if prev is not None:
                                flush(prev)
                            prev = (ats, ktp)
                        flush(prev)
                        fill()

                        for i in (0, 1):
                            h = 2 * hp + i
                            r1 = work.tile([1, 512], bf16, tag="r1", name="r1",
                                           bufs=2)
                            rb = work.tile([64, 512], bf16, tag="rb", name="rb",
                                           bufs=2)
                            with nc.allow_low_precision(reason="softmax scale"):
                                nc.vector.reciprocal(r1[:], psO[i][64:65, :])
                                nc.gpsimd.partition_broadcast(rb[:], r1[:])
                                dsl = slice((h // 2) * T + sp * 512,
                                            (h // 2) * T + (sp + 1) * 512)
                                nc.vector.tensor_mul(
                                    OT2[(h % 2) * 64:(h % 2) * 64 + 64, dsl],
                                    psO[i][0:64, :], rb[:])
                    while fidx[0] < len(fillers):
                        f = fillers[fidx[0]]
                        fidx[0] += 1
                        if f is not None:
                            f()

                # ---- schedule ----
                # startup: projections on the free "s" ring (no eo serial
                # chain), v-quads 0-1 (x half0) dense on PE during rope.
                pe_q0 = proj_mm(0, 0, tag="s", bufs=2)
                rope_ev(0, pe_q0, QDh, QDl)
                pe_k0 = proj_mm(0, 256, tag="s", bufs=2)
                v_quad(0)
                rope_ev(0, pe_k0, KDh, KDl)

                def mk_proj_fillers(sp, c0, DH, DL):
                    box = {}

                    def fa():
                        box["pe"] = ps.tile([128, 1024], f32, tag="eo",
                                            name="psEO", bufs=1)
                        proj_half(sp, c0, box["pe"], 0)
                        box["t"] = rope_a(sp, box["pe"])

                    def fb():
                        proj_half(sp, c0, box["pe"], 1)
                        t1, t3 = box["t"]
                        rope_b(sp, box["pe"], DH, DL, t1, t3)
                    return fa, fb

                q1a, q1b = mk_proj_fillers(1, 0, QDh, QDl)
                k1a, k1b = mk_proj_fillers(1, 256, KDh, KDl)
                q2a, q2b = mk_proj_fillers(2, 0, QDh, QDl)
                k2a, k2b = mk_proj_fillers(2, 256, KDh, KDl)
                q3a, q3b = mk_proj_fillers(3, 0, QDh, QDl)
                k3a, k3b = mk_proj_fillers(3, 256, KDh, KDl)

                attn_span(0, [lambda: v_quad(1), q1a, q1b, k1a, k1b])
                attn_span(1, [q2a, q2b, k2a, k2b,
                              lambda: wo_tt(0, 0), lambda: wo_tt(0, 1),
                              lambda: wo_tt(0, 2), lambda: wo_tt(0, 3),
                              lambda: v_quad(2), lambda: rs_chunk(0)])
                attn_span(2, [q3a, q3b, k3a, k3b,
                              lambda: wo_tt(1, 0), lambda: wo_tt(1, 1),
                              lambda: wo_tt(1, 2), lambda: wo_tt(1, 3),
                              lambda: v_quad(3), lambda: rs_chunk(1)])
                attn_span(3, [lambda: wo_tt(2, 0), lambda: wo_tt(2, 1),
                              lambda: wo_tt(2, 2), lambda: wo_tt(2, 3),
                              lambda: rs_chunk(2)])
                for ttl in range(4):
                    wo_tt(3, ttl)
                rs_chunk(3)

    # Rebase any remaining tile-level waits off dual-fp8 ldweights onto a
    # PE EventSemaphore inserted just before them.
    for blk in nc.main_func.blocks:
        out_insts = []
        for inst in blk.instructions:
            if isinstance(inst, mybir.InstLdweights):
                si = inst.sync_info
                if si is not None and len(si.on_wait) > 0:
                    waits = list(si.on_wait)
                    si.on_wait = []
                    for w0 in range(0, len(waits), 2):
                        ev = mybir.InstEventSemaphore(
                            name=nc.get_next_instruction_name(), ins=[],
                            outs=[])
                        ev.engine = inst.engine
                        ev.sync_info = mybir.SyncInfo(
                            on_wait=waits[w0:w0 + 2], on_update=[])
                        nc.register_instruction(ev)
                        out_insts.append(ev)
            out_insts.append(inst)
        blk.instructions[:] = out_insts
    nc.compile()
    return nc


def _host_tables():
    bf = ml_dtypes.bfloat16
    f8 = ml_dtypes.float8_e4m3
    j = np.arange(0, D, 2, dtype=np.float64)
    inv = 1.0 / (10000.0 ** (j / D))
    t = np.arange(T, dtype=np.float64)
    fr = np.outer(t, inv)                      # [T, 32]
    cosT = np.tile(np.cos(fr).T, (4, 1)) / 16.0  # [128, T]
    sinT = np.tile(np.sin(fr).T, (4, 1)) / 16.0
    csT = np.concatenate([cosT, sinT], axis=1).astype(bf)  # [128, 2T]
    k = np.arange(128)[:, None]
    c = np.arange(128)[None, :]
    band = (c >= k).astype(f8)                 # [128, 128]
    return csT, band


def _in_maps(x, Wq, Wk, Wv, Wo):
    bf = ml_dtypes.bfloat16
    f8 = ml_dtypes.float8_e4m3
    csT, band = _host_tables()
    maps = []
    for core in range(NCORES):
        b = core // 4
        g0 = HLOC * (core % 4)
        heads = range(g0, g0 + HLOC)
        evens = np.concatenate([g * 64 + np.arange(0, 64, 2) for g in heads])
        odds = np.concatenate([g * 64 + np.arange(1, 64, 2) for g in heads])
        perm = np.concatenate([evens, odds])
        vrows = np.concatenate([np.arange(g * 64, (g + 1) * 64) for g in heads])
        wqkv = np.concatenate(
            [Wq[perm].T * 256.0, Wk[perm].T * 256.0, Wv[vrows].T * 16.0],
            axis=1).astype(np.float32)          # [C, 3*CLOC]
        wh = wqkv.astype(f8)
        wl = (wqkv - wh.astype(np.float32)).astype(f8)
        xT = np.ascontiguousarray(x[b].T).astype(np.float32)
        xh = xT.astype(f8)
        xl = (xT - xh.astype(np.float32)).astype(f8)

        def ileave_w(whi, wlo):
            # -> [512, 3072]: row p*128+q blocks
            # [s2p-hi(768) | s2p-lo | s2p+1-hi | s2p+1-lo]
            h4 = whi.reshape(4, 2, 128, 768)
            l4 = wlo.reshape(4, 2, 128, 768)
            out = np.empty((4, 128, 2, 2, 768), whi.dtype)
            out[:, :, :, 0] = h4.transpose(0, 2, 1, 3)
            out[:, :, :, 1] = l4.transpose(0, 2, 1, 3)
            return np.ascontiguousarray(out.reshape(512, 3072))

        def ileave_x(xhi, xlo):
            # -> [512, 8192]: per (p, half): [i0-hi | i0-lo | i1-hi | i1-lo]
            h6 = xhi.reshape(4, 2, 128, 2, 1024).transpose(0, 2, 3, 1, 4)
            l6 = xlo.reshape(4, 2, 128, 2, 1024).transpose(0, 2, 3, 1, 4)
            out = np.empty((4, 128, 2, 2, 2, 1024), xhi.dtype)
            out[:, :, :, :, 0] = h6
            out[:, :, :, :, 1] = l6
            return np.ascontiguousarray(out.reshape(512, 8192))

        wo2 = (Wo[:, vrows].T / 16.0).astype(np.float32)  # [256, C]
        wo2 = np.concatenate([wo2[0:128], wo2[128:256]], axis=1)  # [128, 2C]
        maps.append({
            "xT": ileave_x(xh, xl),
            "wT": ileave_w(wh, wl),
            "woT": np.ascontiguousarray(wo2).astype(bf),
            "csT": csT, "band": band,
        })
    return maps


def _run(x, Wq, Wk, Wv, Wo, trace=False):
    from concourse.bass_utils import run_bass_kernel_spmd

    if "nc" not in _CACHE:
        _CACHE["nc"] = _build_nc()
    nc = _CACHE["nc"]
    maps = _in_maps(x, Wq, Wk, Wv, Wo)
    return run_bass_kernel_spmd(nc, maps, list(range(NCORES)), trace=trace)


def kernel(x, Wq, Wk, Wv, Wo):
    x = np.asarray(x, dtype=np.float32)
    res = _run(x, np.asarray(Wq, np.float32), np.asarray(Wk, np.float32),
               np.asarray(Wv, np.float32), np.asarray(Wo, np.float32))
    y = np.zeros((B, T, C), np.float32)
    for core in range(NCORES):
        b, r = core // 4, core % 4
        o = np.asarray(res.results[core]["out"]).astype(np.float32)
        for c in range(4):
            y[b, c * 512 + r * 128:c * 512 + (r + 1) * 128] = \
                o[c * 128:(c + 1) * 128]
    return y


# revision 4
# speedup vs baseline: 1.0001x; 1.0001x over previous
"""Distributed Trainium2 kernel for AdvancedMultiHeadAttention (fp8 DoubleRow).

B=2, T=2048, C=1024, H=16 heads, D=64. Causal SDPA with RoPE.
Sharding: data-parallel over batch (cores 0-3 = batch 0, 4-7 = batch 1),
tensor-parallel over heads within each group (4 heads/core). Wo is
row-sharded; partial sums are reduced with four bf16 ReduceScatters per
4-core replica group (one per 512-query span, overlapped), host concat.

Numerics: big matmuls run as fp8e4m3 DoubleRow (2x128 contraction/pass,
0.5 cyc/col). Projections and S=QK^T use hi+lo residual splitting (3
terms, error ~fp8^2); AV uses single-fp8 attention weights with hi+lo V
(2 terms). Wo stays bf16. End-to-end rel err ~1.3e-2 (tol 2e-2).

Scales: Wq,Wk x256 on host, cos/sin tables /16 -> Q,K stored as 16*q_rot.
exp(scale=1/2048, bias=-3ln2) -> at = e^u/8 in [0,30] (fp8e4 max 240).
Wv x16 -> V stored 16*v; OT = 16*attn_out; Wo/16 on host -> psY = y.

Scheduling: projection / V-projection / Wo matmul groups are interleaved
into the attention kt loop as fillers so the PE never sits behind the
Act-engine softmax. PSUM: tag "eo" (2 banks, proj+V), tag "s" (4 banks,
S-pairs + Wo), o0/o1 (2 banks, AV accumulators for one head-pair).

Dual-fp8 LdWeights ISA restrictions handled: no semaphore waits on
ldweights (wait-move pass disabled, PE fuse_nops disabled, waits hoisted
to EventSemaphore), and pair-dim stride of every DR stationary AP is
16-byte aligned (V tile padded to 68 cols/head).
"""

import sys

sys.path.insert(0, "/opt/trn_rl_repo")

import math

import ml_dtypes
import numpy as np

B, T, C = 2, 2048, 1024
H, D = 16, 64
NCORES = 8
HLOC = 4            # heads per core
CLOC = HLOC * D     # 256 attention dims per core
NSPAN = T // 512    # 4 query spans
VP = 68             # padded V columns per head (pair stride 4*68 % 16 == 0)
RG = [[0, 1, 2, 3], [4, 5, 6, 7]]

_CACHE = {}


def _build_nc(rs=True, reps=1):
    import concourse.bacc as bacc
    import concourse.mybir as mybir
    import concourse.tile as tile

    f32 = mybir.dt.float32
    bf16 = mybir.dt.bfloat16
    fp8 = mybir.dt.float8e4
    AF = mybir.ActivationFunctionType
    DR = mybir.MatmulPerfMode.DoubleRow
    ALU = mybir.AluOpType
    ESC = 0.125 / 256.0
    EBIAS = -3.0 * math.log(2.0)

    nc = bacc.Bacc("TRN2", target_bir_lowering=False, debug=False,
                   num_devices=NCORES)
    # Dual-fp8 (DoubleRow) InstLdweights cannot carry semaphore waits
    # (walrus s3_lw_dual_fp8_restrictions). Keep waits off ldweights.
    nc.move_matmul_waits_to_ldweights = lambda: None
    _orig_fuse_nops = type(nc).fuse_nops
    nc.fuse_nops = (lambda engine: None if engine == mybir.EngineType.PE
                    else _orig_fuse_nops(nc, engine))

    xT = nc.declare_dram_parameter("xT", [512, 8192], fp8, isOutput=False)
    wT = nc.declare_dram_parameter("wT", [512, 3072], fp8, isOutput=False)
    woT = nc.declare_dram_parameter("woT", [128, 2 * C], bf16, isOutput=False)
    csT = nc.declare_dram_parameter("csT", [128, 2 * T], bf16, isOutput=False)
    bandT = nc.declare_dram_parameter("band", [128, 128], fp8, isOutput=False)
    outx = nc.declare_dram_parameter("out", [512, C], bf16, isOutput=True)

    with tile.TileContext(nc) as tc:
        with (
            tc.tile_pool(name="cst", bufs=1) as cst,
            tc.tile_pool(name="work", bufs=1) as work,
            tc.tile_pool(name="ps", bufs=1, space="PSUM") as ps,
            tc.tile_pool(name="dram", bufs=1, space="DRAM") as dram,
        ):
            # ---- constant loads: merged hi/lo tiles, one DMA each ----
            w2 = [cst.tile([128, 2 * 1536], fp8, tag=f"w2{p}", name=f"w2{p}")
                  for p in range(4)]
            x2 = [[cst.tile([128, 2 * 2048], fp8, tag=f"x2{p}_{s}",
                            name=f"x2{p}_{s}") for s in range(2)]
                  for p in range(4)]
            cs_sb = cst.tile([128, 2 * T], bf16, tag="cs", name="cs")
            for p in range(4):
                r = slice(p * 128, (p + 1) * 128)
                nc.sync.dma_start(w2[p][:], wT[r, :])
                nc.sync.dma_start(x2[p][0][:], xT[r, 0:4096])
            # cos/sin for span 0 right behind the first projection's data
            for spc in range(4):
                nc.sync.dma_start(cs_sb[:, spc * 512:(spc + 1) * 512],
                                  csT[:, spc * 512:(spc + 1) * 512])
                nc.sync.dma_start(cs_sb[:, T + spc * 512:T + (spc + 1) * 512],
                                  csT[:, T + spc * 512:T + (spc + 1) * 512])
            band_sb = cst.tile([128, 128], fp8, tag="band", name="band")
            nc.sync.dma_start(band_sb[:], bandT[:, :])
            for p in range(4):
                r = slice(p * 128, (p + 1) * 128)
                nc.sync.dma_start(x2[p][1][:], xT[r, 4096:8192])
            wo2_sb = cst.tile([128, 2 * C], bf16, tag="wo2", name="wo2")
            nc.sync.dma_start(wo2_sb[:], woT[:, :])
            wo_sb = [wo2_sb[:, i * C:(i + 1) * C] for i in range(2)]

            cos_sb = cs_sb[:, 0:T]
            sin_sb = cs_sb[:, T:2 * T]
            eb = cst.tile([128, 1], f32, tag="eb", name="eb")
            nc.vector.memset(eb[:], EBIAS)

            def w3(lo, p, c0, n):
                ofs = 768 if lo else 0
                return w2[p][:].rearrange(
                    "p (i c) -> p i c", i=2)[:, :, ofs + c0:ofs + c0 + n]

            def x3(lo, p, half, c0, n):
                ofs = 1024 if lo else 0
                return x2[p][half][:].rearrange(
                    "p (i c) -> p i c", i=2)[:, :, ofs + c0:ofs + c0 + n]

            for _rep in range(reps):
                QDh = work.tile([128, 2 * T], fp8, tag="qdh", name="qdh")
                QDl = work.tile([128, 2 * T], fp8, tag="qdl", name="qdl")
                KDh = work.tile([128, 2 * T], fp8, tag="kdh", name="kdh")
                KDl = work.tile([128, 2 * T], fp8, tag="kdl", name="kdl")
                OT2 = work.tile([128, 2 * T], bf16, tag="ot2", name="ot2")
                vah = [work.tile([128, 2 * HLOC * VP], fp8, tag=f"vah{p}",
                                 name=f"vah{p}") for p in range(8)]
                val = [work.tile([128, 2 * HLOC * VP], fp8, tag=f"val{p}",
                                 name=f"val{p}") for p in range(8)]

                def d3(tile_, h, c0, n):
                    return tile_[h * 32:(h + 1) * 32].rearrange(
                        "p (i t) -> p i t", i=2)[:, :, c0:c0 + n]

                def proj_half(sp, c0, pe, hx):
                    cofs = c0 + hx * 128
                    dst = pe[:, hx * 512:(hx + 1) * 512]
                    mm = 0
                    for p in range(4):
                        lw_h = w3(False, p, cofs, 128)
                        lw_l = w3(True, p, cofs, 128)
                        rx_h = x3(False, p, sp // 2, (sp % 2) * 512, 512)
                        rx_l = x3(True, p, sp // 2, (sp % 2) * 512, 512)
                        for lw, rx in ((lw_h, rx_h), (lw_h, rx_l),
                                       (lw_l, rx_h)):
                            nc.tensor.matmul(dst, lw, rx, start=(mm == 0),
                                             stop=(mm == 11), perf_mode=DR)
                            mm += 1

                def proj_mm(sp, c0, tag="eo", bufs=1):
                    """QKV projection matmuls for q (c0=0) or k (c0=256)."""
                    pe = ps.tile([128, 1024], f32, tag=tag, name="psEO",
                                 bufs=bufs)
                    proj_half(sp, c0, pe, 0)
                    proj_half(sp, c0, pe, 1)
                    return pe

                def rope_a(sp, pe):
                    qs = slice(sp * 512, (sp + 1) * 512)
                    psE = pe[:, 0:512]
                    t1 = work.tile([128, 512], bf16, tag="t1", name="t1", bufs=3)
                    t3 = work.tile([128, 512], bf16, tag="t3", name="t3", bufs=3)
                    with nc.allow_low_precision(reason="rope in bf16/fp8"):
                        nc.vector.tensor_mul(t1[:], psE, cos_sb[:, qs])
                        nc.vector.tensor_mul(t3[:], psE, sin_sb[:, qs])
                    return t1, t3

                def rope_b(sp, pe, DH, DL, t1, t3):
                    qs = slice(sp * 512, (sp + 1) * 512)
                    psO = pe[:, 512:1024]
                    t2 = work.tile([128, 512], bf16, tag="t2", name="t2", bufs=3)
                    t4 = work.tile([128, 512], bf16, tag="t4", name="t4", bufs=3)
                    TOP = work.tile([128, 512], bf16, tag="top", name="top",
                                    bufs=3)
                    BOT = work.tile([128, 512], bf16, tag="bot", name="bot",
                                    bufs=3)
                    with nc.allow_low_precision(reason="rope in bf16/fp8"):
                        nc.vector.tensor_mul(t2[:], psO, sin_sb[:, qs])
                        nc.vector.tensor_sub(TOP[:], t1[:], t2[:])
                        nc.vector.tensor_mul(t4[:], psO, cos_sb[:, qs])
                        nc.vector.tensor_add(BOT[:], t3[:], t4[:])
                        for blk, SRC in ((0, TOP), (1, BOT)):
                            csl = slice(blk * T + sp * 512,
                                        blk * T + (sp + 1) * 512)
                            nc.gpsimd.tensor_copy(DH[:, csl], SRC[:])
                            nc.vector.scalar_tensor_tensor(
                                DL[:, csl], DH[:, csl], -1.0, SRC[:],
                                ALU.mult, ALU.add)

                def rope_ev(sp, pe, DH, DL):
                    t1, t3 = rope_a(sp, pe)
                    rope_b(sp, pe, DH, DL, t1, t3)

                def v_quad(quad, tag="eo", bufs=1, evict_dve=False):
                    """V projection for tts 4q..4q+3 -> vaug pairs 2q, 2q+1."""
                    pv = ps.tile([128, 1024], f32, tag=tag, name="psV",
                                 bufs=bufs)
                    for pr in (quad * 2, quad * 2 + 1):
                        with nc.allow_low_precision(reason="ones col"):
                            o_h = vah[pr][:].rearrange("p (i d) -> p i d", i=8)
                            o_l = val[pr][:].rearrange("p (i d) -> p i d", i=8)
                            nc.gpsimd.memset(o_h[:, :, 64:65], 1.0)
                            nc.gpsimd.memset(o_l[:, :, 64:65], 0.0)
                    for ttl in range(4):
                        tt = quad * 4 + ttl
                        dst = pv[:, ttl * 256:(ttl + 1) * 256]
                        mm = 0
                        for p in range(4):
                            lx_h = x3(False, p, tt // 8, (tt % 8) * 128, 128)
                            lx_l = x3(True, p, tt // 8, (tt % 8) * 128, 128)
                            rw_h = w3(False, p, 512, 256)
                            rw_l = w3(True, p, 512, 256)
                            for lx, rw in ((lx_h, rw_h), (lx_l, rw_h),
                                           (lx_h, rw_l)):
                                nc.tensor.matmul(dst, lx, rw, start=(mm == 0),
                                                 stop=(mm == 11), perf_mode=DR)
                                mm += 1
                        pr = quad * 2 + ttl // 2
                        i = tt % 2
                        src = dst.rearrange("p (h d) -> p h d", h=HLOC)
                        vh_dst = vah[pr][:].rearrange(
                            "p (i h d) -> p i h d", i=2, h=HLOC)[:, i, :, 0:64]
                        vl_dst = val[pr][:].rearrange(
                            "p (i h d) -> p i h d", i=2, h=HLOC)[:, i, :, 0:64]
                        with nc.allow_low_precision(reason="v fp8 hi/lo"):
                            nc.scalar.copy(vh_dst, src)
                            nc.vector.scalar_tensor_tensor(
                                vl_dst, vh_dst, -1.0, src, ALU.mult, ALU.add)

                def wo_tt(c, ttl):
                    yb = _ybs[c]
                    if True:
                        tt = c * 4 + ttl
                        psY = ps.tile([128, 1024], f32,
                                      tag=("eo" if c < 3 else "s"), name="psY",
                                      bufs=(1 if c < 3 else 2))
                        for cs in range(2):
                            for s2 in range(2):
                                nc.tensor.matmul(
                                    psY[:, cs * 512:(cs + 1) * 512],
                                    OT2[:, s2 * T + tt * 128:
                                        s2 * T + (tt + 1) * 128],
                                    wo_sb[s2][:, cs * 512:(cs + 1) * 512],
                                    start=(s2 == 0), stop=(s2 == 1))
                        ysb = work.tile([128, C], bf16, tag="ysb", name="ysb",
                                        bufs=2)
                        nc.scalar.copy(ysb[:], psY[:])
                        nc.sync.dma_start(yb[ttl * 128:(ttl + 1) * 128, :],
                                          ysb[:])

                def rs_chunk(c):
                    import concourse.mybir as mybir_
                    yb = _ybs[c]
                    if rs:
                        rst = dram.tile([128, C], bf16, tag=f"rs{c}_{_rep}",
                                        name=f"rs{c}_{_rep}")
                        nc.gpsimd.collective_compute(
                            "ReduceScatter", mybir_.AluOpType.add,
                            replica_groups=RG,
                            ins=[yb[:].opt()], outs=[rst[:].opt()])
                        nc.sync.dma_start(outx[c * 128:(c + 1) * 128, :],
                                          rst[:])
                    else:
                        core_r = 0
                        nc.sync.dma_start(outx[c * 128:(c + 1) * 128, :],
                                          yb[core_r * 128:(core_r + 1) * 128, :])

                _ybs = [dram.tile([512, C], bf16, tag=f"yb{c}_{_rep}",
                                  name=f"yb{c}_{_rep}") for c in range(4)]

                def attn_span(sp, fillers):
                    qs = slice(sp * 512, (sp + 1) * 512)
                    npair = 2 * (sp + 1)
                    fidx = [0]

                    def fill():
                        if fidx[0] < len(fillers):
                            f = fillers[fidx[0]]
                            fidx[0] += 1
                            if f is not None:
                                f()

                    for hp in (0, 1):
                        psO = [ps.tile([65, 512], f32, tag=f"o{i}",
                                       name=f"psAcc{i}") for i in (0, 1)]
                        pending = []

                        def flush(prev):
                            ats, pktp = prev
                            for i in (0, 1):
                                h = 2 * hp + i
                                rhs = ats[i][:].rearrange("p (i t) -> p i t",
                                                          i=2)
                                lw_h = vah[pktp][:].rearrange(
                                    "p (i h d) -> p i h d", i=2,
                                    h=HLOC)[:, :, h, 0:65]
                                lw_l = val[pktp][:].rearrange(
                                    "p (i h d) -> p i h d", i=2,
                                    h=HLOC)[:, :, h, 0:65]
                                nc.tensor.matmul(psO[i][:], lw_h, rhs,
                                                 start=(pktp == 0), stop=False,
                                                 perf_mode=DR)
                                nc.tensor.matmul(psO[i][:], lw_l, rhs,
                                                 start=False,
                                                 stop=(pktp == npair - 1),
                                                 perf_mode=DR)

                        for ktp in range(npair):
                            ats = []
                            for i in (0, 1):
                                h = 2 * hp + i
                                psS = ps.tile([128, 1024], f32, tag="s",
                                              name=f"psS{i}", bufs=2)
                                for j, kt in enumerate((2 * ktp, 2 * ktp + 1)):
                                    dst = psS[:, j * 512:(j + 1) * 512]
                                    kh = d3(KDh, h, kt * 128, 128)
                                    kl = d3(KDl, h, kt * 128, 128)
                                    qh = d3(QDh, h, sp * 512, 512)
                                    ql = d3(QDl, h, sp * 512, 512)
                                    for mm, (lk, rq) in enumerate(
                                            ((kh, qh), (kh, ql), (kl, qh))):
                                        nc.tensor.matmul(
                                            dst, lk, rq, start=(mm == 0),
                                            stop=(mm == 2), perf_mode=DR,
                                            tile_position=(h * 32, 0))
                                at = work.tile([128, 1024], fp8, tag=f"at{i}",
                                               name=f"at{i}", bufs=8)
                                if ktp < 2 * sp:
                                    nc.scalar.activation(at[:], psS[:], AF.Exp,
                                                         scale=ESC, bias=eb[:])
                                else:
                                    for j in (0, 1):
                                        d = (2 * ktp + j) * 128 - sp * 512
                                        c0 = j * 512
                                        if d > 0:
                                            nc.gpsimd.memset(
                                                at[:, c0:c0 + d], 0.0)
                                        nc.scalar.activation(
                                            at[:, c0 + d:c0 + 512],
                                            psS[:, c0 + d:c0 + 512],
                                            AF.Exp, scale=ESC, bias=eb[:])
                                        with nc.allow_low_precision(
                                                reason="mask"):
                                            nc.gpsimd.tensor_mul(
                                                at[:, c0 + d:c0 + d + 128],
                                                at[:, c0 + d:c0 + d + 128],
                                                band_sb[:])
                                ats.append(at)
                            fill()
                            pending.append((ats, ktp))
                            if len(pending) > 2:
                                flush(pending.pop(0))
                        for item in pending:
                            flush(item)
                        fill()

                        for i in (0, 1):
                            h = 2 * hp + i
                            r1 = work.tile([1, 512], bf16, tag="r1", name="r1",
                                           bufs=2)
                            rb = work.tile([64, 512], bf16, tag="rb", name="rb",
                                           bufs=2)
                            with nc.allow_low_precision(reason="softmax scale"):
                                nc.vector.reciprocal(r1[:], psO[i][64:65, :])
                                nc.gpsimd.partition_broadcast(rb[:], r1[:])
                                dsl = slice((h // 2) * T + sp * 512,
                                            (h // 2) * T + (sp + 1) * 512)
                                nc.vector.tensor_mul(
                                    OT2[(h % 2) * 64:(h % 2) * 64 + 64, dsl],
                                    psO[i][0:64, :], rb[:])
                    while fidx[0] < len(fillers):
                        f = fillers[fidx[0]]
                        fidx[0] += 1
                        if f is not None:
                            f()

                # ---- schedule ----
                # startup: projections on the free "s" ring (no eo serial
                # chain), v-quads 0-1 (x half0) dense on PE during rope.
                pe_q0 = proj_mm(0, 0, tag="s", bufs=2)
                rope_ev(0, pe_q0, QDh, QDl)
                pe_k0 = proj_mm(0, 256, tag="s", bufs=2)
                v_quad(0)
                rope_ev(0, pe_k0, KDh, KDl)

                def mk_proj_fillers(sp, c0, DH, DL):
                    box = {}

                    def fa():
                        box["pe"] = ps.tile([128, 1024], f32, tag="eo",
                                            name="psEO", bufs=1)
                        proj_half(sp, c0, box["pe"], 0)
                        box["t"] = rope_a(sp, box["pe"])

                    def fb():
                        proj_half(sp, c0, box["pe"], 1)
                        t1, t3 = box["t"]
                        rope_b(sp, box["pe"], DH, DL, t1, t3)
                    return fa, fb

                q1a, q1b = mk_proj_fillers(1, 0, QDh, QDl)
                k1a, k1b = mk_proj_fillers(1, 256, KDh, KDl)
                q2a, q2b = mk_proj_fillers(2, 0, QDh, QDl)
                k2a, k2b = mk_proj_fillers(2, 256, KDh, KDl)
                q3a, q3b = mk_proj_fillers(3, 0, QDh, QDl)
                k3a, k3b = mk_proj_fillers(3, 256, KDh, KDl)

                attn_span(0, [lambda: v_quad(1), q1a, q1b, k1a, k1b])
                attn_span(1, [q2a, q2b, k2a, k2b,
                              lambda: wo_tt(0, 0), lambda: wo_tt(0, 1),
                              lambda: wo_tt(0, 2), lambda: wo_tt(0, 3),
                              lambda: v_quad(2), lambda: rs_chunk(0)])
                attn_span(2, [q3a, q3b, k3a, k3b,
                              lambda: wo_tt(1, 0), lambda: wo_tt(1, 1),
                              lambda: wo_tt(1, 2), lambda: wo_tt(1, 3),
                              lambda: v_quad(3), lambda: rs_chunk(1)])
                attn_span(3, [lambda: wo_tt(2, 0), lambda: wo_tt(2, 1),
                              lambda: wo_tt(2, 2), lambda: wo_tt(2, 3),
                              lambda: rs_chunk(2)])
                for ttl in range(4):
                    wo_tt(3, ttl)
                rs_chunk(3)

    # Rebase any remaining tile-level waits off dual-fp8 ldweights onto a
    # PE EventSemaphore inserted just before them.
    for blk in nc.main_func.blocks:
        out_insts = []
        for inst in blk.instructions:
            if isinstance(inst, mybir.InstLdweights):
                si = inst.sync_info
                if si is not None and len(si.on_wait) > 0:
                    waits = list(si.on_wait)
                    si.on_wait = []
                    for w0 in range(0, len(waits), 2):
                        ev = mybir.InstEventSemaphore(
                            name=nc.get_next_instruction_name(), ins=[],
                            outs=[])
                        ev.engine = inst.engine
                        ev.sync_info = mybir.SyncInfo(
                            on_wait=waits[w0:w0 + 2], on_update=[])
                        nc.register_instruction(ev)
                        out_insts.append(ev)
            out_insts.append(inst)
        blk.instructions[:] = out_insts
    nc.compile()
    return nc


def _host_tables():
    bf = ml_dtypes.bfloat16
    f8 = ml_dtypes.float8_e4m3
    j = np.arange(0, D, 2, dtype=np.float64)
    inv = 1.0 / (10000.0 ** (j / D))
    t = np.arange(T, dtype=np.float64)
    fr = np.outer(t, inv)                      # [T, 32]
    cosT = np.tile(np.cos(fr).T, (4, 1)) / 16.0  # [128, T]
    sinT = np.tile(np.sin(fr).T, (4, 1)) / 16.0
    csT = np.concatenate([cosT, sinT], axis=1).astype(bf)  # [128, 2T]
    k = np.arange(128)[:, None]
    c = np.arange(128)[None, :]
    band = (c >= k).astype(f8)                 # [128, 128]
    return csT, band


def _in_maps(x, Wq, Wk, Wv, Wo):
    bf = ml_dtypes.bfloat16
    f8 = ml_dtypes.float8_e4m3
    csT, band = _host_tables()
    maps = []
    for core in range(NCORES):
        b = core // 4
        g0 = HLOC * (core % 4)
        heads = range(g0, g0 + HLOC)
        evens = np.concatenate([g * 64 + np.arange(0, 64, 2) for g in heads])
        odds = np.concatenate([g * 64 + np.arange(1, 64, 2) for g in heads])
        perm = np.concatenate([evens, odds])
        vrows = np.concatenate([np.arange(g * 64, (g + 1) * 64) for g in heads])
        wqkv = np.concatenate(
            [Wq[perm].T * 256.0, Wk[perm].T * 256.0, Wv[vrows].T * 16.0],
            axis=1).astype(np.float32)          # [C, 3*CLOC]
        wh = wqkv.astype(f8)
        wl = (wqkv - wh.astype(np.float32)).astype(f8)
        xT = np.ascontiguousarray(x[b].T).astype(np.float32)
        xh = xT.astype(f8)
        xl = (xT - xh.astype(np.float32)).astype(f8)

        def ileave_w(whi, wlo):
            # -> [512, 3072]: row p*128+q blocks
            # [s2p-hi(768) | s2p-lo | s2p+1-hi | s2p+1-lo]
            h4 = whi.reshape(4, 2, 128, 768)
            l4 = wlo.reshape(4, 2, 128, 768)
            out = np.empty((4, 128, 2, 2, 768), whi.dtype)
            out[:, :, :, 0] = h4.transpose(0, 2, 1, 3)
            out[:, :, :, 1] = l4.transpose(0, 2, 1, 3)
            return np.ascontiguousarray(out.reshape(512, 3072))

        def ileave_x(xhi, xlo):
            # -> [512, 8192]: per (p, half): [i0-hi | i0-lo | i1-hi | i1-lo]
            h6 = xhi.reshape(4, 2, 128, 2, 1024).transpose(0, 2, 3, 1, 4)
            l6 = xlo.reshape(4, 2, 128, 2, 1024).transpose(0, 2, 3, 1, 4)
            out = np.empty((4, 128, 2, 2, 2, 1024), xhi.dtype)
            out[:, :, :, :, 0] = h6
            out[:, :, :, :, 1] = l6
            return np.ascontiguousarray(out.reshape(512, 8192))

        wo2 = (Wo[:, vrows].T / 16.0).astype(np.float32)  # [256, C]
        wo2 = np.concatenate([wo2[0:128], wo2[128:256]], axis=1)  # [128, 2C]
        maps.append({
            "xT": ileave_x(xh, xl),
            "wT": ileave_w(wh, wl),
            "woT": np.ascontiguousarray(wo2).astype(bf),
            "csT": csT, "band": band,
        })
    return maps


def _run(x, Wq, Wk, Wv, Wo, trace=False):
    from concourse.bass_utils import run_bass_kernel_spmd

    if "nc" not in _CACHE:
        _CACHE["nc"] = _build_nc()
    nc = _CACHE["nc"]
    maps = _in_maps(x, Wq, Wk, Wv, Wo)
    return run_bass_kernel_spmd(nc, maps, list(range(NCORES)), trace=trace)


def kernel(x, Wq, Wk, Wv, Wo):
    x = np.asarray(x, dtype=np.float32)
    res = _run(x, np.asarray(Wq, np.float32), np.asarray(Wk, np.float32),
               np.asarray(Wv, np.float32), np.asarray(Wo, np.float32))
    y = np.zeros((B, T, C), np.float32)
    for core in range(NCORES):
        b, r = core // 4, core % 4
        o = np.asarray(res.results[core]["out"]).astype(np.float32)
        for c in range(4):
            y[b, c * 512 + r * 128:c * 512 + (r + 1) * 128] = \
                o[c * 128:(c + 1) * 128]
    return y


# revision 5
# speedup vs baseline: 1.0164x; 1.0163x over previous
"""Distributed Trainium2 kernel for AdvancedMultiHeadAttention (fp8 DoubleRow).

B=2, T=2048, C=1024, H=16 heads, D=64. Causal SDPA with RoPE.
Sharding: data-parallel over batch (cores 0-3 = batch 0, 4-7 = batch 1),
tensor-parallel over heads within each group (4 heads/core). Wo is
row-sharded; partial sums are reduced with four bf16 ReduceScatters per
4-core replica group (one per 512-query span, overlapped), host concat.

Numerics: big matmuls run as fp8e4m3 DoubleRow (2x128 contraction/pass,
0.5 cyc/col). Projections and S=QK^T use hi+lo residual splitting (3
terms, error ~fp8^2); AV uses single-fp8 attention weights with hi+lo V
(2 terms). Wo stays bf16. End-to-end rel err ~1.3e-2 (tol 2e-2).

Scales: Wq,Wk x256 on host, cos/sin tables /16 -> Q,K stored as 16*q_rot.
exp(scale=1/2048, bias=-3ln2) -> at = e^u/8 in [0,30] (fp8e4 max 240).
Wv x16 -> V stored 16*v; OT = 16*attn_out; Wo/16 on host -> psY = y.

Scheduling: projection / V-projection / Wo matmul groups are interleaved
into the attention kt loop as fillers so the PE never sits behind the
Act-engine softmax. PSUM: tag "eo" (2 banks, proj+V), tag "s" (4 banks,
S-pairs + Wo), o0/o1 (2 banks, AV accumulators for one head-pair).

Dual-fp8 LdWeights ISA restrictions handled: no semaphore waits on
ldweights (wait-move pass disabled, PE fuse_nops disabled, waits hoisted
to EventSemaphore), and pair-dim stride of every DR stationary AP is
16-byte aligned (V tile padded to 68 cols/head).
"""

import sys

sys.path.insert(0, "/opt/trn_rl_repo")

import math

import ml_dtypes
import numpy as np

B, T, C = 2, 2048, 1024
H, D = 16, 64
NCORES = 8
HLOC = 4            # heads per core
CLOC = HLOC * D     # 256 attention dims per core
NSPAN = T // 512    # 4 query spans
VP = 68             # padded V columns per head (pair stride 4*68 % 16 == 0)
RG = [[0, 1, 2, 3], [4, 5, 6, 7]]

_CACHE = {}


def _build_nc(rs=True, reps=1):
    import concourse.bacc as bacc
    import concourse.mybir as mybir
    import concourse.tile as tile

    f32 = mybir.dt.float32
    bf16 = mybir.dt.bfloat16
    fp8 = mybir.dt.float8e4
    AF = mybir.ActivationFunctionType
    DR = mybir.MatmulPerfMode.DoubleRow
    ALU = mybir.AluOpType
    ESC = 0.125 / 256.0
    EBIAS = -3.0 * math.log(2.0)

    nc = bacc.Bacc("TRN2", target_bir_lowering=False, debug=False,
                   num_devices=NCORES)
    # Dual-fp8 (DoubleRow) InstLdweights cannot carry semaphore waits
    # (walrus s3_lw_dual_fp8_restrictions). Keep waits off ldweights.
    nc.move_matmul_waits_to_ldweights = lambda: None
    _orig_fuse_nops = type(nc).fuse_nops
    nc.fuse_nops = (lambda engine: None if engine == mybir.EngineType.PE
                    else _orig_fuse_nops(nc, engine))

    xT = nc.declare_dram_parameter("xT", [512, 8192], fp8, isOutput=False)
    wT = nc.declare_dram_parameter("wT", [512, 3072], fp8, isOutput=False)
    woT = nc.declare_dram_parameter("woT", [128, 2 * C], bf16, isOutput=False)
    csT = nc.declare_dram_parameter("csT", [128, 2 * T], bf16, isOutput=False)
    bandT = nc.declare_dram_parameter("band", [128, 128], fp8, isOutput=False)
    outx = nc.declare_dram_parameter("out", [512, C], bf16, isOutput=True)

    with tile.TileContext(nc) as tc:
        with (
            tc.tile_pool(name="cst", bufs=1) as cst,
            tc.tile_pool(name="work", bufs=1) as work,
            tc.tile_pool(name="ps", bufs=1, space="PSUM") as ps,
            tc.tile_pool(name="dram", bufs=1, space="DRAM") as dram,
        ):
            # ---- constant loads: merged hi/lo tiles, one DMA each ----
            w2 = [cst.tile([128, 2 * 1536], fp8, tag=f"w2{p}", name=f"w2{p}")
                  for p in range(4)]
            x2 = [[cst.tile([128, 2 * 2048], fp8, tag=f"x2{p}_{s}",
                            name=f"x2{p}_{s}") for s in range(2)]
                  for p in range(4)]
            cs_sb = cst.tile([128, 2 * T], bf16, tag="cs", name="cs")
            for p in range(4):
                r = slice(p * 128, (p + 1) * 128)
                nc.sync.dma_start(w2[p][:], wT[r, :])
                nc.sync.dma_start(x2[p][0][:], xT[r, 0:4096])
            # cos/sin for span 0 right behind the first projection's data
            for spc in range(4):
                nc.sync.dma_start(cs_sb[:, spc * 512:(spc + 1) * 512],
                                  csT[:, spc * 512:(spc + 1) * 512])
                nc.sync.dma_start(cs_sb[:, T + spc * 512:T + (spc + 1) * 512],
                                  csT[:, T + spc * 512:T + (spc + 1) * 512])
            band_sb = cst.tile([128, 128], fp8, tag="band", name="band")
            nc.sync.dma_start(band_sb[:], bandT[:, :])
            for p in range(4):
                r = slice(p * 128, (p + 1) * 128)
                nc.sync.dma_start(x2[p][1][:], xT[r, 4096:8192])
            wo2_sb = cst.tile([128, 2 * C], bf16, tag="wo2", name="wo2")
            nc.sync.dma_start(wo2_sb[:], woT[:, :])
            wo_sb = [wo2_sb[:, i * C:(i + 1) * C] for i in range(2)]

            cos_sb = cs_sb[:, 0:T]
            sin_sb = cs_sb[:, T:2 * T]
            eb = cst.tile([128, 1], f32, tag="eb", name="eb")
            nc.vector.memset(eb[:], EBIAS)

            def w3(lo, p, c0, n):
                ofs = 768 if lo else 0
                return w2[p][:].rearrange(
                    "p (i c) -> p i c", i=2)[:, :, ofs + c0:ofs + c0 + n]

            def x3(lo, p, half, c0, n):
                ofs = 1024 if lo else 0
                return x2[p][half][:].rearrange(
                    "p (i c) -> p i c", i=2)[:, :, ofs + c0:ofs + c0 + n]

            for _rep in range(reps):
                QDh = work.tile([128, 2 * T], fp8, tag="qdh", name="qdh")
                QDl = work.tile([128, 2 * T], fp8, tag="qdl", name="qdl")
                KDh = work.tile([128, 2 * T], fp8, tag="kdh", name="kdh")
                KDl = work.tile([128, 2 * T], fp8, tag="kdl", name="kdl")
                OT2 = work.tile([128, 2 * T], bf16, tag="ot2", name="ot2")
                vah = [work.tile([128, 2 * HLOC * VP], fp8, tag=f"vah{p}",
                                 name=f"vah{p}") for p in range(8)]
                val = [work.tile([128, 2 * HLOC * VP], fp8, tag=f"val{p}",
                                 name=f"val{p}") for p in range(8)]

                def d3(tile_, h, c0, n):
                    return tile_[h * 32:(h + 1) * 32].rearrange(
                        "p (i t) -> p i t", i=2)[:, :, c0:c0 + n]

                def proj_half(sp, c0, pe, hx):
                    cofs = c0 + hx * 128
                    dst = pe[:, hx * 512:(hx + 1) * 512]
                    mm = 0
                    for p in range(4):
                        lw_h = w3(False, p, cofs, 128)
                        lw_l = w3(True, p, cofs, 128)
                        rx_h = x3(False, p, sp // 2, (sp % 2) * 512, 512)
                        rx_l = x3(True, p, sp // 2, (sp % 2) * 512, 512)
                        for lw, rx in ((lw_h, rx_h), (lw_h, rx_l),
                                       (lw_l, rx_h)):
                            nc.tensor.matmul(dst, lw, rx, start=(mm == 0),
                                             stop=(mm == 11), perf_mode=DR)
                            mm += 1

                def proj_mm(sp, c0, tag="eo", bufs=1):
                    """QKV projection matmuls for q (c0=0) or k (c0=256)."""
                    pe = ps.tile([128, 1024], f32, tag=tag, name="psEO",
                                 bufs=bufs)
                    proj_half(sp, c0, pe, 0)
                    proj_half(sp, c0, pe, 1)
                    return pe

                def rope_a(sp, pe):
                    qs = slice(sp * 512, (sp + 1) * 512)
                    psE = pe[:, 0:512]
                    t1 = work.tile([128, 512], bf16, tag="t1", name="t1", bufs=3)
                    t3 = work.tile([128, 512], bf16, tag="t3", name="t3", bufs=3)
                    with nc.allow_low_precision(reason="rope in bf16/fp8"):
                        nc.vector.tensor_mul(t1[:], psE, cos_sb[:, qs])
                        nc.vector.tensor_mul(t3[:], psE, sin_sb[:, qs])
                    return t1, t3

                def rope_b(sp, pe, DH, DL, t1, t3):
                    qs = slice(sp * 512, (sp + 1) * 512)
                    psO = pe[:, 512:1024]
                    t2 = work.tile([128, 512], bf16, tag="t2", name="t2", bufs=3)
                    t4 = work.tile([128, 512], bf16, tag="t4", name="t4", bufs=3)
                    TOP = work.tile([128, 512], bf16, tag="top", name="top",
                                    bufs=3)
                    BOT = work.tile([128, 512], bf16, tag="bot", name="bot",
                                    bufs=3)
                    with nc.allow_low_precision(reason="rope in bf16/fp8"):
                        nc.vector.tensor_mul(t2[:], psO, sin_sb[:, qs])
                        nc.vector.tensor_sub(TOP[:], t1[:], t2[:])
                        nc.vector.tensor_mul(t4[:], psO, cos_sb[:, qs])
                        nc.vector.tensor_add(BOT[:], t3[:], t4[:])
                        for blk, SRC in ((0, TOP), (1, BOT)):
                            csl = slice(blk * T + sp * 512,
                                        blk * T + (sp + 1) * 512)
                            nc.gpsimd.tensor_copy(DH[:, csl], SRC[:])
                            nc.vector.scalar_tensor_tensor(
                                DL[:, csl], DH[:, csl], -1.0, SRC[:],
                                ALU.mult, ALU.add)

                def rope_ev(sp, pe, DH, DL):
                    t1, t3 = rope_a(sp, pe)
                    rope_b(sp, pe, DH, DL, t1, t3)

                def v_quad(quad, tag="eo", bufs=1, evict_dve=False):
                    """V projection for tts 4q..4q+3 -> vaug pairs 2q, 2q+1."""
                    pv = ps.tile([128, 1024], f32, tag=tag, name="psV",
                                 bufs=bufs)
                    for pr in (quad * 2, quad * 2 + 1):
                        with nc.allow_low_precision(reason="ones col"):
                            o_h = vah[pr][:].rearrange("p (i d) -> p i d", i=8)
                            o_l = val[pr][:].rearrange("p (i d) -> p i d", i=8)
                            nc.gpsimd.memset(o_h[:, :, 64:65], 1.0)
                            nc.gpsimd.memset(o_l[:, :, 64:65], 0.0)
                    for ttl in range(4):
                        tt = quad * 4 + ttl
                        dst = pv[:, ttl * 256:(ttl + 1) * 256]
                        mm = 0
                        for p in range(4):
                            lx_h = x3(False, p, tt // 8, (tt % 8) * 128, 128)
                            lx_l = x3(True, p, tt // 8, (tt % 8) * 128, 128)
                            rw_h = w3(False, p, 512, 256)
                            rw_l = w3(True, p, 512, 256)
                            for lx, rw in ((lx_h, rw_h), (lx_l, rw_h),
                                           (lx_h, rw_l)):
                                nc.tensor.matmul(dst, lx, rw, start=(mm == 0),
                                                 stop=(mm == 11), perf_mode=DR)
                                mm += 1
                        pr = quad * 2 + ttl // 2
                        i = tt % 2
                        src = dst.rearrange("p (h d) -> p h d", h=HLOC)
                        vh_dst = vah[pr][:].rearrange(
                            "p (i h d) -> p i h d", i=2, h=HLOC)[:, i, :, 0:64]
                        vl_dst = val[pr][:].rearrange(
                            "p (i h d) -> p i h d", i=2, h=HLOC)[:, i, :, 0:64]
                        with nc.allow_low_precision(reason="v fp8 hi/lo"):
                            nc.scalar.copy(vh_dst, src)
                            nc.vector.scalar_tensor_tensor(
                                vl_dst, vh_dst, -1.0, src, ALU.mult, ALU.add)

                def wo_tt(c, ttl):
                    yb = _ybs[c]
                    if True:
                        tt = c * 4 + ttl
                        psY = ps.tile([128, 1024], f32,
                                      tag=("eo" if c < 3 else "s"), name="psY",
                                      bufs=(1 if c < 3 else 2))
                        for cs in range(2):
                            for s2 in range(2):
                                nc.tensor.matmul(
                                    psY[:, cs * 512:(cs + 1) * 512],
                                    OT2[:, s2 * T + tt * 128:
                                        s2 * T + (tt + 1) * 128],
                                    wo_sb[s2][:, cs * 512:(cs + 1) * 512],
                                    start=(s2 == 0), stop=(s2 == 1))
                        ysb = work.tile([128, C], bf16, tag="ysb", name="ysb",
                                        bufs=2)
                        if c < 3 and ttl % 2 == 1:
                            nc.vector.tensor_copy(ysb[:], psY[:])
                        else:
                            nc.scalar.copy(ysb[:], psY[:])
                        nc.sync.dma_start(yb[ttl * 128:(ttl + 1) * 128, :],
                                          ysb[:])

                def rs_chunk(c):
                    import concourse.mybir as mybir_
                    yb = _ybs[c]
                    if rs:
                        rst = dram.tile([128, C], bf16, tag=f"rs{c}_{_rep}",
                                        name=f"rs{c}_{_rep}")
                        nc.gpsimd.collective_compute(
                            "ReduceScatter", mybir_.AluOpType.add,
                            replica_groups=RG,
                            ins=[yb[:].opt()], outs=[rst[:].opt()])
                        nc.sync.dma_start(outx[c * 128:(c + 1) * 128, :],
                                          rst[:])
                    else:
                        core_r = 0
                        nc.sync.dma_start(outx[c * 128:(c + 1) * 128, :],
                                          yb[core_r * 128:(core_r + 1) * 128, :])

                _ybs = [dram.tile([512, C], bf16, tag=f"yb{c}_{_rep}",
                                  name=f"yb{c}_{_rep}") for c in range(4)]

                def attn_span(sp, fillers):
                    qs = slice(sp * 512, (sp + 1) * 512)
                    npair = 2 * (sp + 1)
                    fidx = [0]

                    def fill():
                        if fidx[0] < len(fillers):
                            f = fillers[fidx[0]]
                            fidx[0] += 1
                            if f is not None:
                                f()

                    for hp in (0, 1):
                        psO = [ps.tile([65, 512], f32, tag=f"o{i}",
                                       name=f"psAcc{i}") for i in (0, 1)]
                        pending = []

                        def flush(prev):
                            ats, pktp = prev
                            for i in (0, 1):
                                h = 2 * hp + i
                                rhs = ats[i][:].rearrange("p (i t) -> p i t",
                                                          i=2)
                                lw_h = vah[pktp][:].rearrange(
                                    "p (i h d) -> p i h d", i=2,
                                    h=HLOC)[:, :, h, 0:65]
                                lw_l = val[pktp][:].rearrange(
                                    "p (i h d) -> p i h d", i=2,
                                    h=HLOC)[:, :, h, 0:65]
                                nc.tensor.matmul(psO[i][:], lw_h, rhs,
                                                 start=(pktp == 0), stop=False,
                                                 perf_mode=DR)
                                nc.tensor.matmul(psO[i][:], lw_l, rhs,
                                                 start=False,
                                                 stop=(pktp == npair - 1),
                                                 perf_mode=DR)

                        for ktp in range(npair):
                            ats = []
                            for i in (0, 1):
                                h = 2 * hp + i
                                psS = ps.tile([128, 1024], f32, tag="s",
                                              name=f"psS{i}", bufs=2)
                                for j, kt in enumerate((2 * ktp, 2 * ktp + 1)):
                                    dst = psS[:, j * 512:(j + 1) * 512]
                                    kh = d3(KDh, h, kt * 128, 128)
                                    kl = d3(KDl, h, kt * 128, 128)
                                    qh = d3(QDh, h, sp * 512, 512)
                                    ql = d3(QDl, h, sp * 512, 512)
                                    for mm, (lk, rq) in enumerate(
                                            ((kh, qh), (kh, ql), (kl, qh))):
                                        nc.tensor.matmul(
                                            dst, lk, rq, start=(mm == 0),
                                            stop=(mm == 2), perf_mode=DR,
                                            tile_position=(h * 32, 0))
                                at = work.tile([128, 1024], fp8, tag=f"at{i}",
                                               name=f"at{i}", bufs=8)
                                if ktp < 2 * sp:
                                    nc.scalar.activation(at[:], psS[:], AF.Exp,
                                                         scale=ESC, bias=eb[:])
                                else:
                                    for j in (0, 1):
                                        d = (2 * ktp + j) * 128 - sp * 512
                                        c0 = j * 512
                                        if d > 0:
                                            nc.gpsimd.memset(
                                                at[:, c0:c0 + d], 0.0)
                                        nc.scalar.activation(
                                            at[:, c0 + d:c0 + 512],
                                            psS[:, c0 + d:c0 + 512],
                                            AF.Exp, scale=ESC, bias=eb[:])
                                        with nc.allow_low_precision(
                                                reason="mask"):
                                            nc.gpsimd.tensor_mul(
                                                at[:, c0 + d:c0 + d + 128],
                                                at[:, c0 + d:c0 + d + 128],
                                                band_sb[:])
                                ats.append(at)
                            fill()
                            pending.append((ats, ktp))
                            if len(pending) > 2:
                                flush(pending.pop(0))
                        for item in pending:
                            flush(item)
                        fill()

                        for i in (0, 1):
                            h = 2 * hp + i
                            r1 = work.tile([1, 512], bf16, tag="r1", name="r1",
                                           bufs=2)
                            rb = work.tile([64, 512], bf16, tag="rb", name="rb",
                                           bufs=2)
                            with nc.allow_low_precision(reason="softmax scale"):
                                nc.vector.reciprocal(r1[:], psO[i][64:65, :])
                                nc.gpsimd.partition_broadcast(rb[:], r1[:])
                                dsl = slice((h // 2) * T + sp * 512,
                                            (h // 2) * T + (sp + 1) * 512)
                                nc.vector.tensor_mul(
                                    OT2[(h % 2) * 64:(h % 2) * 64 + 64, dsl],
                                    psO[i][0:64, :], rb[:])
                    while fidx[0] < len(fillers):
                        f = fillers[fidx[0]]
                        fidx[0] += 1
                        if f is not None:
                            f()

                # ---- schedule ----
                # startup: projections on the free "s" ring (no eo serial
                # chain), v-quads 0-1 (x half0) dense on PE during rope.
                pe_q0 = proj_mm(0, 0, tag="s", bufs=2)
                rope_ev(0, pe_q0, QDh, QDl)
                pe_k0 = proj_mm(0, 256, tag="s", bufs=2)
                v_quad(0)
                rope_ev(0, pe_k0, KDh, KDl)

                def mk_proj_fillers(sp, c0, DH, DL):
                    box = {}

                    def fa():
                        box["pe"] = ps.tile([128, 1024], f32, tag="eo",
                                            name="psEO", bufs=1)
                        proj_half(sp, c0, box["pe"], 0)
                        box["t"] = rope_a(sp, box["pe"])

                    def fb():
                        proj_half(sp, c0, box["pe"], 1)
                        t1, t3 = box["t"]
                        rope_b(sp, box["pe"], DH, DL, t1, t3)
                    return fa, fb

                q1a, q1b = mk_proj_fillers(1, 0, QDh, QDl)
                k1a, k1b = mk_proj_fillers(1, 256, KDh, KDl)
                q2a, q2b = mk_proj_fillers(2, 0, QDh, QDl)
                k2a, k2b = mk_proj_fillers(2, 256, KDh, KDl)
                q3a, q3b = mk_proj_fillers(3, 0, QDh, QDl)
                k3a, k3b = mk_proj_fillers(3, 256, KDh, KDl)

                attn_span(0, [lambda: v_quad(1), q1a, q1b, k1a, k1b])
                attn_span(1, [q2a, q2b, k2a, k2b,
                              lambda: wo_tt(0, 0), lambda: wo_tt(0, 1),
                              lambda: wo_tt(0, 2), lambda: wo_tt(0, 3),
                              lambda: v_quad(2), lambda: rs_chunk(0)])
                attn_span(2, [q3a, q3b, k3a, k3b,
                              lambda: wo_tt(1, 0), lambda: wo_tt(1, 1),
                              lambda: wo_tt(1, 2), lambda: wo_tt(1, 3),
                              lambda: v_quad(3), lambda: rs_chunk(1)])
                attn_span(3, [lambda: wo_tt(2, 0), lambda: wo_tt(2, 1),
                              lambda: wo_tt(2, 2), lambda: wo_tt(2, 3),
                              lambda: rs_chunk(2)])
                for ttl in range(4):
                    wo_tt(3, ttl)
                rs_chunk(3)

    # Rebase any remaining tile-level waits off dual-fp8 ldweights onto a
    # PE EventSemaphore inserted just before them.
    for blk in nc.main_func.blocks:
        out_insts = []
        for inst in blk.instructions:
            if isinstance(inst, mybir.InstLdweights):
                si = inst.sync_info
                if si is not None and len(si.on_wait) > 0:
                    waits = list(si.on_wait)
                    si.on_wait = []
                    for w0 in range(0, len(waits), 2):
                        ev = mybir.InstEventSemaphore(
                            name=nc.get_next_instruction_name(), ins=[],
                            outs=[])
                        ev.engine = inst.engine
                        ev.sync_info = mybir.SyncInfo(
                            on_wait=waits[w0:w0 + 2], on_update=[])
                        nc.register_instruction(ev)
                        out_insts.append(ev)
            out_insts.append(inst)
        blk.instructions[:] = out_insts
    nc.compile()
    return nc


def _host_tables():
    bf = ml_dtypes.bfloat16
    f8 = ml_dtypes.float8_e4m3
    j = np.arange(0, D, 2, dtype=np.float64)
    inv = 1.0 / (10000.0 ** (j / D))
    t = np.arange(T, dtype=np.float64)
    fr = np.outer(t, inv)                      # [T, 32]
    cosT = np.tile(np.cos(fr).T, (4, 1)) / 16.0  # [128, T]
    sinT = np.tile(np.sin(fr).T, (4, 1)) / 16.0
    csT = np.concatenate([cosT, sinT], axis=1).astype(bf)  # [128, 2T]
    k = np.arange(128)[:, None]
    c = np.arange(128)[None, :]
    band = (c >= k).astype(f8)                 # [128, 128]
    return csT, band


def _in_maps(x, Wq, Wk, Wv, Wo):
    bf = ml_dtypes.bfloat16
    f8 = ml_dtypes.float8_e4m3
    csT, band = _host_tables()
    maps = []
    for core in range(NCORES):
        b = core // 4
        g0 = HLOC * (core % 4)
        heads = range(g0, g0 + HLOC)
        evens = np.concatenate([g * 64 + np.arange(0, 64, 2) for g in heads])
        odds = np.concatenate([g * 64 + np.arange(1, 64, 2) for g in heads])
        perm = np.concatenate([evens, odds])
        vrows = np.concatenate([np.arange(g * 64, (g + 1) * 64) for g in heads])
        wqkv = np.concatenate(
            [Wq[perm].T * 256.0, Wk[perm].T * 256.0, Wv[vrows].T * 16.0],
            axis=1).astype(np.float32)          # [C, 3*CLOC]
        wh = wqkv.astype(f8)
        wl = (wqkv - wh.astype(np.float32)).astype(f8)
        xT = np.ascontiguousarray(x[b].T).astype(np.float32)
        xh = xT.astype(f8)
        xl = (xT - xh.astype(np.float32)).astype(f8)

        def ileave_w(whi, wlo):
            # -> [512, 3072]: row p*128+q blocks
            # [s2p-hi(768) | s2p-lo | s2p+1-hi | s2p+1-lo]
            h4 = whi.reshape(4, 2, 128, 768)
            l4 = wlo.reshape(4, 2, 128, 768)
            out = np.empty((4, 128, 2, 2, 768), whi.dtype)
            out[:, :, :, 0] = h4.transpose(0, 2, 1, 3)
            out[:, :, :, 1] = l4.transpose(0, 2, 1, 3)
            return np.ascontiguousarray(out.reshape(512, 3072))

        def ileave_x(xhi, xlo):
            # -> [512, 8192]: per (p, half): [i0-hi | i0-lo | i1-hi | i1-lo]
            h6 = xhi.reshape(4, 2, 128, 2, 1024).transpose(0, 2, 3, 1, 4)
            l6 = xlo.reshape(4, 2, 128, 2, 1024).transpose(0, 2, 3, 1, 4)
            out = np.empty((4, 128, 2, 2, 2, 1024), xhi.dtype)
            out[:, :, :, :, 0] = h6
            out[:, :, :, :, 1] = l6
            return np.ascontiguousarray(out.reshape(512, 8192))

        wo2 = (Wo[:, vrows].T / 16.0).astype(np.float32)  # [256, C]
        wo2 = np.concatenate([wo2[0:128], wo2[128:256]], axis=1)  # [128, 2C]
        maps.append({
            "xT": ileave_x(xh, xl),
            "wT": ileave_w(wh, wl),
            "woT": np.ascontiguousarray(wo2).astype(bf),
            "csT": csT, "band": band,
        })
    return maps


def _run(x, Wq, Wk, Wv, Wo, trace=False):
    from concourse.bass_utils import run_bass_kernel_spmd

    if "nc" not in _CACHE:
        _CACHE["nc"] = _build_nc()
    nc = _CACHE["nc"]
    maps = _in_maps(x, Wq, Wk, Wv, Wo)
    return run_bass_kernel_spmd(nc, maps, list(range(NCORES)), trace=trace)


def kernel(x, Wq, Wk, Wv, Wo):
    x = np.asarray(x, dtype=np.float32)
    res = _run(x, np.asarray(Wq, np.float32), np.asarray(Wk, np.float32),
               np.asarray(Wv, np.float32), np.asarray(Wo, np.float32))
    y = np.zeros((B, T, C), np.float32)
    for core in range(NCORES):
        b, r = core // 4, core % 4
        o = np.asarray(res.results[core]["out"]).astype(np.float32)
        for c in range(4):
            y[b, c * 512 + r * 128:c * 512 + (r + 1) * 128] = \
                o[c * 128:(c + 1) * 128]
    return y
